# revision 48
# baseline (speedup 1.0000x reference)
"""Wilson-Dirac operator on Trainium2, 8 NeuronCores, T-axis domain decomposition.

v4: DVE + TensorE hybrid. The baseline (v2) ran everything on the DVE
(99% busy, 627us). v4 keeps only the per-site work that genuinely needs a
two-tensor elementwise engine on the DVE (spin projection `proj`, SU(3)
component products `products`, and one half-reduction add per direction),
and moves ALL constant-coefficient linear algebra (color reduction over a,
the complex re/im combination, the spin expansion out[s] += d*m[e], the
y-shift of the forward-y hop term, and the mass term) onto the idle
TensorEngine as accumulating identity/permutation-weight matmuls into PSUM:

    out_psum[(s,ri) region][row, c*S + t*Z + z]  +=  sigma * P[row, plane-AP]

with lhsT in {+I, -I, +Yshift, -Yshift, 4.5*I} (contract over the partition
dim = lattice rows; the plane selection/z-shift/swizzle lives in the moving
operand's AP). Each (s,ri) region is 3*S = 432 fp32 = one PSUM bank; the
24-plane output accumulator occupies exactly the 8 banks. ScalarE (also
idle in v2) evacuates PSUM fp32 -> SBUF fp16 per region, then the result is
DMA'd out. fp32 PSUM accumulation also improves precision vs the fp16
RMW chain of v2.

Per direction the PE consumes P via 4 accumulating matmuls per (s,ri)
region: 2 signed (th,tu)-groups x 2 partial sums {P01 = a0+a1 (DVE), Pa2}.
DIRS_NOBSUM directions skip the DVE half-bsum and use 6 matmuls/region
(3 a-terms per group); the set is tuned so DVE busy ~= PE busy (~320us
each, overlapped to a ~360us wall vs the 628us all-DVE baseline).

Shift handling: t via inline halos; x via DRAM row reloads; BOTH y hops
via permutation weights on the PE (fwd: out rows shifted; bwd: host ships
a y-rolled gauge slot and the PE shifts the product rows back) - no shift
DMAs remain; z-fwd via a z-offset in the PE moving AP (split main+wrap
matmuls); z-bwd via an h-plane copy on the otherwise idle ScalarE.

Other scheduling tricks: psi/gauge/x-row loads prefetched one x-block
ahead (the sync queue otherwise stalls loads behind the out-DMA, which
waits on the PSUM evacuation); 48 junk matmuls at startup spin the PE
through the HAM activity window so real matmuls run at 2.4 GHz; the last
block orders PE-heavy dirs first to shrink the end-of-kernel PE backlog;
P tiles triple-buffered so the DVE can run ~3 dirs ahead of the PE."""

import numpy as np

# ---------------------------------------------------------------- constants
X = Y = 24
Z = 24
T = 48
NCORES = 8
TS = T // NCORES          # 6 t-slices per core
TH = TS + 2               # psi t slots (with halo both sides)
TG = TS + 1               # gauge t slots (halo at t0-1)
S = TS * Z                # 144: work-plane span [t,z]
PP = TH * Z               # 192: psi plane span
GP = TG * Z               # 168: gauge plane span
XY = X * Y
MASSP4 = 4.5
RMAX = 120                # max rows per x-block (NXC*Y)

# h_j = psi[j] + c_j * psi[B_j]; expansion: out[0]+=m[0], out[1]+=m[1],
# out[2] += d0*m[e0], out[3] += d1*m[e1].  Backward: c -> -c, d -> -d.
DIRSPEC = {
    0: dict(B=(3, 2), c=(-1j, -1j), e=(1, 0), d=(+1j, +1j)),
    1: dict(B=(3, 2), c=(-1, +1),   e=(1, 0), d=(+1, -1)),
    2: dict(B=(2, 3), c=(-1j, +1j), e=(0, 1), d=(+1j, -1j)),
    3: dict(B=(2, 3), c=(+1, +1),   e=(0, 1), d=(+1, +1)),
}

# processing order; x-dirs last so their DRAM row loads prefetch; (2,-1)
# delayed so its ScalarE h z-shift (queued behind the previous block's
# evacuations) has latency headroom.
DIRS_ORDER = [(2, +1), (3, +1), (3, -1), (2, -1),
              (1, +1), (1, -1), (0, +1), (0, -1)]
# last block: PE-heavy dirs first so the PE backlog at kernel end is small
# (the tail is PE finishing after the DVE's last op).
DIRS_ORDER_LAST = [(2, +1), (3, +1), (3, -1), (2, -1),
                   (0, +1), (1, +1), (1, -1), (0, -1)]
# dirs whose a-reduction happens fully on the PE (6 matmuls/region instead
# of 4) to offload the DVE.  Tuned against engine-busy split.
DIRS_NOBSUM = {(3, +1), (3, -1), (0, +1), (2, -1)}

_CACHE = {}


def _split_waits_json(raw: bytes) -> bytes:
    """Walrus here allows only ONE sync-wait per instruction. Keep the last
    wait on the instruction, hoist the rest onto NoOps inserted immediately
    before it (same engine, semaphores monotonic => exact)."""
    import json
    bj = json.loads(raw)
    nid = 0
    for fn in bj.get("functions", []):
        for bb in fn.get("blocks", []):
            out = []
            changed = False
            for inst in bb.get("instructions", []):
                si = inst.get("sync_info")
                ow = (si or {}).get("on_wait") or []
                if len(ow) > 1:
                    changed = True
                    for w in ow[:-1]:
                        nid += 1
                        out.append({
                            "engine": inst["engine"], "ins": [], "outs": [],
                            "name": f"WSPL-{nid}", "opcode": "NoOp",
                            "sync_info": {"on_update": [], "on_wait": [w]},
                        })
                    si["on_wait"] = [ow[-1]]
                out.append(inst)
            if changed:
                bb["instructions"] = out
    return json.dumps(bj).encode()


def _install_json_wait_fix():
    import concourse.bass as bass
    if getattr(bass.Bass, "_wd_wait_fix", False):
        return
    orig = bass.Bass.to_json_bytes

    def patched(self, *a, **k):
        return _split_waits_json(orig(self, *a, **k))

    bass.Bass.to_json_bytes = patched
    bass.Bass._wd_wait_fix = True


def _dir_terms(mu, sgn):
    """Per (s, ri) output region: list of (sigma, j, th, tu) matmul terms.

    m_re[j] = P[j,0,0] + f*P[j,1,1];  m_im[j] = P[j,1,0] - f*P[j,0,1]
    (f = +1 fwd / -1 bwd; P[j,th,tu] is the a-summed product with h-part th
    and U-part tu).  Regions (s,ri):
      s in {0,1}: += m[j=s, ri]
      s = 2+si:   j = e[si], dv = +-d[si]:
        dv real:  += sign(dv) * m[j, ri]
        dv imag:  region ri=0 += -sg*m_im[j]; region ri=1 += +sg*m_re[j]
    """
    spec = DIRSPEC[mu]
    fwd = sgn > 0
    f = 1 if fwd else -1
    out = {}

    def m_terms(j, ri, sigma):
        if ri == 0:   # m_re
            return [(sigma, j, 0, 0), (sigma * f, j, 1, 1)]
        else:         # m_im
            return [(sigma, j, 1, 0), (-sigma * f, j, 0, 1)]

    for s in (0, 1):
        for ri in (0, 1):
            out[(s, ri)] = m_terms(s, ri, +1)
    for si in (0, 1):
        s = 2 + si
        j = spec["e"][si]
        dv = spec["d"][si] if fwd else -spec["d"][si]
        if dv.imag == 0.0:
            sigma = 1 if dv.real > 0 else -1
            for ri in (0, 1):
                out[(s, ri)] = m_terms(j, ri, sigma)
        else:
            sg = 1 if dv.imag > 0 else -1
            out[(s, 0)] = m_terms(j, 1, -sg)
            out[(s, 1)] = m_terms(j, 0, +sg)
    return out


def build_module(NXC=5):
    import concourse.bass as bass
    import concourse.mybir as mybir
    from concourse.ap import AP
    from concourse.mybir import AluOpType
    from concourse.tile import TileContext

    _install_json_wait_fix()

    F16 = mybir.dt.float16
    F32 = mybir.dt.float32

    nc = bass.Bass()
    fh = nc.declare_dram_parameter("fh", [XY, 24 * PP], F16, isOutput=False)
    # gauge packed row-major: per row the 5 slots [mu0..mu3, mu1 y-rolled]
    # so one DMA per block fetches all local gauge.
    gg = nc.declare_dram_parameter("gg", [XY, 5 * 18 * GP], F16, isOutput=False)
    gx = nc.declare_dram_parameter("gx", [XY, 18 * GP], F16, isOutput=False)
    # weight matrices [RMAX, RMAX]: [I+, I-, Yf+, Yf-, 4.5I, Yb+, Yb-]
    wc = nc.declare_dram_parameter("wc", [RMAX, 7 * RMAX], F16, isOutput=False)
    outp = nc.declare_dram_parameter("outp", [XY, 24 * S], F16, isOutput=True)

    def sap(t, off, dims):
        return AP(t.tensor, t.offset + off, [list(t.ap[0])] + [list(d) for d in dims])

    def psap(t, off, dims, rows):
        ap0 = [list(t.ap[0])[0], rows]
        return AP(t.tensor, t.offset + off, [ap0] + [list(d) for d in dims])

    with TileContext(nc) as tc:
        ctx_pool = tc.tile_pool(name="work", bufs=1)
        pool = ctx_pool.__enter__()
        ctx_psum = tc.tile_pool(name="acc", bufs=1, space="PSUM")
        ppool = ctx_psum.__enter__()
        V = nc.vector
        A = AluOpType

        # persistent weight tiles
        wt = pool.tile([RMAX, 7 * RMAX], F16, tag="wc", bufs=1)
        nc.scalar.dma_start(out=wt[:], in_=wc[:])
        warm = {"todo": True}

        def W(idx, rows):
            # lhsT [K=rows, M=rows] slice of weight matrix idx
            return AP(wt.tensor, wt.offset + idx * RMAX,
                      [[list(wt.ap[0])[0], rows], [1, rows]])

        def dma(out, in_):
            nc.sync.dma_start(out=out, in_=in_)

        def load_rows(tag, src, drow, nelem, bufs, base, nrows, eng=None,
                      delay_on=None):
            tl = pool.tile([nrows, nelem], F16, tag=tag, bufs=bufs, name=tag)
            if delay_on is not None:
                # tiny dummy copy: makes this tile's DMA wait (WAW) until
                # `delay_on` has landed, keeping SDMA bandwidth focused on
                # the critical first loads.
                nc.scalar.copy(out=tl[0:1, 0:8], in_=delay_on[0:1, 0:8])
            rs = (base + drow) % XY
            def issue(o, i):
                if eng is None:
                    dma(o, i)
                else:
                    eng(out=o, in_=i)
            if rs + nrows <= XY:
                issue(tl[:], src(rs, rs + nrows))
            else:
                n1 = XY - rs
                issue(tl[0:n1], src(rs, XY))
                issue(tl[n1:nrows], src(0, nrows - n1))
            return tl

        def load_gauge(gl, base, mus, eng=None, delay_on=None):
            nr = min(NXC * Y, XY - base)
            for mu in mus:
                c0 = mu * 18 * GP
                gl[mu] = load_rows(f"g{mu}",
                                   lambda a, b, c=c0: gg[a:b, c:c + 18 * GP],
                                   0, 18 * GP, 2, base, nr, eng,
                                   delay_on=delay_on)
            return gl

        def load_main(base):
            """psi + local gauge slots (separate DMAs, consumption order so
            the first directions' data lands first)."""
            nr = min(NXC * Y, XY - base)
            psi = load_rows("psi", lambda a, b: fh[a:b], 0, 24 * PP, 2, base, nr)
            return psi, load_gauge({}, base, (2, 3, 1, 4, 0))

        def load_xtiles(base, eng=None, delay_on=None):
            nr = min(NXC * Y, XY - base)
            return (
                load_rows("pxf", lambda a, b: fh[a:b], -Y, 24 * PP, 2, base,
                          nr, eng, delay_on=delay_on),
                load_rows("pxb", lambda a, b: fh[a:b], +Y, 24 * PP, 2, base,
                          nr, eng, delay_on=delay_on),
                load_rows("gxf", lambda a, b: gx[a:b], -Y, 18 * GP, 2, base,
                          nr, eng, delay_on=delay_on),
            )

        xpre = None
        mpre = None
        for x0 in range(0, X, NXC):
            nx = min(NXC, X - x0)
            R = nx * Y
            r0 = x0 * Y

            # main + x-shifted loads: prefetched one block ahead so the sync
            # queue issues them before it blocks on the previous block's
            # out-DMA.  Block 0: load ONLY psi+g2 before the first DVE ops
            # (the first DVE instruction conservatively waits on every DMA
            # issued before it); everything else is emitted after front(0).
            xt = {}
            defer_loads = mpre is None
            if defer_loads:
                nr0 = min(NXC * Y, XY - r0)
                psi_al = pool.tile([nr0, 24 * PP], F16, tag="psi", bufs=2,
                                   name="psi")
                h0 = nr0 // 2
                nc.sync.dma_start(out=psi_al[0:h0], in_=fh[r0:r0 + h0])
                nc.scalar.dma_start(out=psi_al[h0:nr0],
                                    in_=fh[r0 + h0:r0 + nr0])
                gl = load_gauge({}, r0, (2,))
            else:
                psi_al, gl = mpre
                xt["v"] = xpre
            dirs = DIRS_ORDER_LAST if x0 + NXC >= X else DIRS_ORDER
            if not defer_loads:
                if x0 + NXC < X:
                    mpre = load_main((x0 + NXC) * Y)
                    xpre = load_xtiles((x0 + NXC) * Y)
                else:
                    mpre = xpre = None

            # PSUM accumulator regions: one bank per (s, ri)
            acc = {}
            for s in range(4):
                for ri in (0, 1):
                    acc[(s, ri)] = ppool.tile([RMAX, 512], F32,
                                              name=f"ps{s}{ri}",
                                              tag=f"ps{s}{ri}", bufs=1)
            # emit matmuls dir-major (PE consumes P tiles as DVE finishes
            # them); start/stop flags from precomputed per-region totals.
            n_per_dir = {}
            for mu_, sg_ in DIRS_ORDER:
                n = (2 if (mu_, sg_) in DIRS_NOBSUM else 0) + 4
                if mu_ == 2 and sg_ == +1:
                    n *= 2                       # z-fwd main+wrap split
                n_per_dir[(mu_, sg_)] = n
            region_total = 1 + sum(n_per_dir.values())   # +1 for mass
            mm_count = {k: 0 for k in acc}

            def emit_mm(key, widx, rhs, oap):
                i = mm_count[key]
                mm_count[key] = i + 1
                nc.tensor.matmul(oap, W(widx, rhs.ap[0][1]), rhs,
                                 start=(i == 0), stop=(i == region_total - 1))

            if warm.pop("todo", False):
                # junk matmuls while the first loads land: spin the PE past
                # the HAM 3.4us activity window so real matmuls run at
                # 2.4 GHz.  Results are discarded - the first real matmul per
                # region has start=True, which clears the bank.
                for w in range(48):
                    reg = acc[(w % 4, (w // 4) % 2)]
                    nc.tensor.matmul(
                        psap(reg, 0, [[1, RMAX]], RMAX),
                        W(0, RMAX), AP(wt.tensor, wt.offset,
                                       [list(wt.ap[0]), [1, RMAX]]),
                        start=True, stop=True, skip_group_check=True)

            # ---------------- op emitters ----------------------------------
            def proj(psi_t, toff, B, cj):
                """h[j,b,ri] = psi[A=j] + c_j psi[B_j]; returns h tile."""
                ht = pool.tile([R, 12 * S], F16, tag="h", bufs=3)
                jB = (B[1] - B[0]) * 6 * PP
                if cj[0] == cj[1] and cj[0].imag == 0.0:
                    op = A.add if cj[0].real > 0 else A.subtract
                    V.tensor_tensor(
                        sap(ht, 0, [[6 * S, 2], [S, 6], [1, S]]),
                        sap(psi_t, toff, [[6 * PP, 2], [PP, 6], [1, S]]),
                        sap(psi_t, B[0] * 6 * PP + toff, [[jB, 2], [PP, 6], [1, S]]),
                        op)
                elif cj[0] == cj[1]:
                    sg = cj[0].imag > 0
                    V.tensor_tensor(
                        sap(ht, 0, [[6 * S, 2], [2 * S, 3], [1, S]]),
                        sap(psi_t, toff, [[6 * PP, 2], [2 * PP, 3], [1, S]]),
                        sap(psi_t, B[0] * 6 * PP + PP + toff,
                            [[jB, 2], [2 * PP, 3], [1, S]]),
                        A.subtract if sg else A.add)
                    V.tensor_tensor(
                        sap(ht, S, [[6 * S, 2], [2 * S, 3], [1, S]]),
                        sap(psi_t, PP + toff, [[6 * PP, 2], [2 * PP, 3], [1, S]]),
                        sap(psi_t, B[0] * 6 * PP + toff,
                            [[jB, 2], [2 * PP, 3], [1, S]]),
                        A.add if sg else A.subtract)
                    return ht
                else:
                    for j in (0, 1):
                        c = cj[j]
                        ab = j * 6 * PP + toff
                        bb_ = B[j] * 6 * PP + toff
                        if c.imag == 0.0:
                            op = A.add if c.real > 0 else A.subtract
                            V.tensor_tensor(
                                sap(ht, j * 6 * S, [[S, 6], [1, S]]),
                                sap(psi_t, ab, [[PP, 6], [1, S]]),
                                sap(psi_t, bb_, [[PP, 6], [1, S]]), op)
                        else:
                            sg = c.imag > 0
                            # h_re = psiA_re -+ psiB_im ; h_im = psiA_im +- psiB_re
                            V.tensor_tensor(
                                sap(ht, j * 6 * S, [[2 * S, 3], [1, S]]),
                                sap(psi_t, ab, [[2 * PP, 3], [1, S]]),
                                sap(psi_t, bb_ + PP, [[2 * PP, 3], [1, S]]),
                                A.subtract if sg else A.add)
                            V.tensor_tensor(
                                sap(ht, j * 6 * S + S, [[2 * S, 3], [1, S]]),
                                sap(psi_t, ab + PP, [[2 * PP, 3], [1, S]]),
                                sap(psi_t, bb_, [[2 * PP, 3], [1, S]]),
                                A.add if sg else A.subtract)
                return ht

            def su3_front(g_t, gtoff, transposed, ht, half_bsum):
                """products (+ optional a0+=a1 half reduction) into P tile."""
                pt = pool.tile([R, 72 * S], F16, tag="P", bufs=3)
                if transposed:
                    gdims = [[3 * GP, 3], [GP, 3], [1, S]]     # read U[b,a]
                else:
                    gdims = [[GP, 3], [3 * GP, 3], [1, S]]     # read U[a,b]
                for j in (0, 1):
                    for th in (0, 1):
                        for tu in (0, 1):
                            V.tensor_tensor(
                                sap(pt, (j * 36 + th * 18 + tu * 9) * S,
                                    [[3 * S, 3], [S, 3], [1, S]]),
                                sap(g_t, tu * 9 * GP + gtoff, gdims),
                                sap(ht, (j * 6 + th) * S, [[2 * S, 3], [0, 3], [1, S]]),
                                A.mult)
                if half_bsum:
                    bdims = [[9 * S, 8], [1, 3 * S]]
                    V.tensor_tensor(sap(pt, 0, bdims), sap(pt, 0, bdims),
                                    sap(pt, 3 * S, bdims), A.add)
                return pt

            def zshift(src_t, nplanes, dz, tag):
                """dst[t,z] = src[t, z+dz] (periodic), dz in {-1,+1}.
                Runs on ScalarE (idle) to keep the DVE free."""
                dt_ = pool.tile([R, nplanes * S], F16, tag="h", bufs=3)
                C = nc.scalar.copy
                if dz == +1:
                    C(sap(dt_, 0, [[S, nplanes], [Z, TS], [1, Z - 1]]),
                      sap(src_t, 1, [[S, nplanes], [Z, TS], [1, Z - 1]]))
                    C(sap(dt_, Z - 1, [[S, nplanes], [Z, TS], [1, 1]]),
                      sap(src_t, 0, [[S, nplanes], [Z, TS], [1, 1]]))
                else:
                    C(sap(dt_, 1, [[S, nplanes], [Z, TS], [1, Z - 1]]),
                      sap(src_t, 0, [[S, nplanes], [Z, TS], [1, Z - 1]]))
                    C(sap(dt_, 0, [[S, nplanes], [Z, TS], [1, 1]]),
                      sap(src_t, Z - 1, [[S, nplanes], [Z, TS], [1, 1]]))
                return dt_

            # ---------------- PE accumulation ------------------------------
            def queue_dir_mms(pt, mu, sgn):
                """emit the per-region accumulating matmuls for direction."""
                terms = _dir_terms(mu, sgn)
                half = (mu, sgn) not in DIRS_NOBSUM
                # y hops: the row shift is a permutation in the weights
                wbase = 2 if (mu, sgn) == (1, +1) else \
                        5 if (mu, sgn) == (1, -1) else 0
                zs = (mu == 2 and sgn == +1)      # z-fwd: shift via rhs AP
                for (s, ri), tl in terms.items():
                    oreg = acc[(s, ri)]
                    for (sigma, j, th, tu) in tl:
                        base = (j * 36 + th * 18 + tu * 9) * S
                        widx = wbase + (0 if sigma > 0 else 1)
                        aoffs = (0, 6 * S) if half else (0, 3 * S, 6 * S)
                        for ao in aoffs:
                            if not zs:
                                rhs = sap(pt, base + ao, [[S, 3], [1, S]])
                                oap = psap(oreg, 0, [[S, 3], [1, S]], R)
                                emit_mm((s, ri), widx, rhs, oap)
                            else:
                                # out[.., z] += sigma * P[.., z-1 (mod Z)]
                                rhs = sap(pt, base + ao, [[Z, 3 * TS], [1, Z - 1]])
                                oap = psap(oreg, 1, [[Z, 3 * TS], [1, Z - 1]], R)
                                emit_mm((s, ri), widx, rhs, oap)
                                rhsw = sap(pt, base + ao + Z - 1,
                                           [[Z, 3 * TS], [1, 1]])
                                oapw = psap(oreg, 0, [[Z, 3 * TS], [1, 1]], R)
                                emit_mm((s, ri), widx, rhsw, oapw)

            def queue_mass():
                for s in range(4):
                    for ri in (0, 1):
                        rhs = sap(psi_al, s * 6 * PP + ri * PP + Z,
                                  [[2 * PP, 3], [1, S]])
                        oap = psap(acc[(s, ri)], 0, [[S, 3], [1, S]], R)
                        emit_mm((s, ri), 4, rhs, oap)

            # ---------------- direction pipeline ---------------------------
            st = {}

            def make_h(i):
                mu, sgn = dirs[i]
                spec = DIRSPEC[mu]
                fwd = sgn == +1
                cj = spec["c"] if fwd else tuple(-v for v in spec["c"])
                if mu == 2:
                    ht = proj(psi_al, Z, spec["B"], cj)
                    if not fwd:
                        ht = zshift(ht, 12, +1, "hsh")
                elif mu == 3:
                    ht = proj(psi_al, 0 if fwd else 2 * Z, spec["B"], cj)
                elif mu == 1:
                    # both y hops computed at the source row; the row shift
                    # happens in the PE weights (Yf/Yb permutations)
                    ht = proj(psi_al, Z, spec["B"], cj)
                else:
                    psi_xf, psi_xb, _ = xt["v"]
                    ht = proj(psi_xf if fwd else psi_xb, Z, spec["B"], cj)
                st[("h", i)] = ht

            def front(i):
                mu, sgn = dirs[i]
                fwd = sgn == +1
                if mu == 2:
                    g_t, gtoff = gl[2], Z
                elif mu == 3:
                    g_t, gtoff = gl[3], (0 if fwd else Z)
                elif mu == 1:
                    # bwd-y reads the host-y-rolled gauge (U at row y-1)
                    g_t, gtoff = (gl[1] if fwd else gl[4]), Z
                else:
                    g_t, gtoff = (xt["v"][2] if fwd else gl[0]), Z
                half = (mu, sgn) not in DIRS_NOBSUM
                pt = su3_front(g_t, gtoff, fwd, st[("h", i)], half)
                queue_dir_mms(pt, mu, sgn)

            # front(0) ASAP so the PE gets work early; make_h(5) hoisted to
            # give its y-shift row DMAs (scalar queue) latency headroom.
            queue_mass()
            done_h = set()

            def ensure_h(i):
                if i not in done_h:
                    make_h(i)
                    done_h.add(i)

            ensure_h(0)
            front(0)
            if defer_loads:
                load_gauge(gl, r0, (3, 1), eng=nc.scalar.dma_start)
                load_gauge(gl, r0, (4, 0), delay_on=psi_al)
                xt["v"] = load_xtiles(r0, delay_on=psi_al)
                if x0 + NXC < X:
                    mpre = load_main((x0 + NXC) * Y)
                    xpre = load_xtiles((x0 + NXC) * Y)
                else:
                    mpre = xpre = None
            ensure_h(1)
            front(1)
            ensure_h(3)   # lead time for the ScalarE h z-shift of (2,-1)
            for i in range(2, 8):
                ensure_h(i)
                front(i)

            # ---------------- evacuate PSUM -> SBUF fp16 -------------------
            out_t = pool.tile([R, 24 * S], F16, tag="out", bufs=1)
            for s in range(4):
                for ri in (0, 1):
                    nc.scalar.copy(
                        out=sap(out_t, (s * 6 + ri) * S, [[2 * S, 3], [1, S]]),
                        in_=psap(acc[(s, ri)], 0, [[S, 3], [1, S]], R))

            nc.sync.dma_start(out=outp[r0:r0 + R], in_=out_t[:])
        ctx_psum.__exit__(None, None, None)
        ctx_pool.__exit__(None, None, None)
    return nc


# ---------------------------------------------------------------- host side
def _prep_core_inputs(fv, gv, t0):
    """fv: [X,Y,Z,T,3,4,2] f32 (c,s,ri). gv: [4,X,Y,Z,T,3,3,2] (r,c,ri).
    Returns fh [XY, 24*(TH*Z)] planes (s,c,ri) layout [t,z], and
    gg [4, XY, 18*(TG*Z)] planes (ri,r,c) of -0.5*U, both fp16."""
    Tl = T
    slots = [(t0 - 1) % Tl] + [(t0 + i) % Tl for i in range(TS)] + [(t0 + TS) % Tl]
    f = fv[:, :, :, slots]                       # [X,Y,Z,TH,c,s,ri]
    f = f.transpose(0, 1, 5, 4, 6, 3, 2)         # [X,Y,s,c,ri,TH,Z]
    fhn = np.ascontiguousarray(f, dtype=np.float16).reshape(XY, 24 * PP)
    gslots = [(t0 - 1 + i) % Tl for i in range(TG)]
    g = gv[:, :, :, :, gslots]                   # [4,X,Y,Z,TG,r,c,ri]
    g = g.transpose(0, 1, 2, 7, 5, 6, 4, 3)      # [4,X,Y,ri,r,c,TG,Z]
    ggn = np.ascontiguousarray(g, dtype=np.float32)
    ggn *= -0.5
    # slot 4: mu1 gauge y-rolled (+1): g1y[x, y] = g1[x, y-1]
    g1y = np.roll(ggn[1], 1, axis=1)
    ggn = np.concatenate([ggn, g1y[None]], axis=0).astype(np.float16)
    ggn = ggn.reshape(5, XY, 18 * GP)
    # pack row-major: [XY, 5 slots x 18*GP]; plus standalone mu0 for x-loads
    ggp = np.ascontiguousarray(ggn.transpose(1, 0, 2)).reshape(XY, 5 * 18 * GP)
    gxn = np.ascontiguousarray(ggn[0])
    return fhn, ggp, gxn


def _make_weights():
    """[RMAX, 7*RMAX] fp16: [I+, I-, Yf+, Yf-, 4.5I, Yb+, Yb-].
    Yf: W[p_in, p_out] = 1 iff p_out = p_in + 1 (mod Y, per x-line):
    out[y] += P[y-1] (fwd-y hop).  Yb: p_out = p_in - 1 (bwd-y hop)."""
    eye = np.eye(RMAX, dtype=np.float16)
    ysh = np.zeros((RMAX, RMAX), dtype=np.float16)
    for g in range(RMAX // Y):
        b = g * Y
        for y in range(Y):
            ysh[b + y, b + (y + 1) % Y] = 1.0
    ybk = ysh.T.copy()
    return np.concatenate([eye, -eye, ysh, -ysh,
                           np.float16(MASSP4) * eye, ybk, -ybk], axis=1)


def _out_to_complex(o):
    o = o.astype(np.float32).reshape(X, Y, 4, 3, 2, TS, Z)   # [X,Y,s,c,ri,t,z]
    o = o.transpose(0, 1, 6, 5, 3, 2, 4)                     # [X,Y,Z,t,c,s,ri]
    return (o[..., 0] + 1j * o[..., 1]).astype(np.complex64)


def _build_in_maps(field, gauge_field):
    fv = np.ascontiguousarray(field).view(np.float32).reshape(X, Y, Z, T, 3, 4, 2)
    gv = np.ascontiguousarray(gauge_field).view(np.float32).reshape(4, X, Y, Z, T, 3, 3, 2)
    wcn = _make_weights()
    in_maps = []
    for k in range(NCORES):
        fhn, ggp, gxn = _prep_core_inputs(fv, gv, k * TS)
        in_maps.append({"fh": fhn, "gg": ggp, "gx": gxn, "wc": wcn})
    return in_maps


def kernel(field, gauge_field):
    from concourse.bass_utils import run_bass_kernel_spmd

    key = "full"
    if key not in _CACHE:
        _CACHE[key] = build_module()
    nc = _CACHE[key]

    in_maps = _build_in_maps(field, gauge_field)
    res = run_bass_kernel_spmd(nc, in_maps, list(range(NCORES))).results

    out = np.empty((X, Y, Z, T, 3, 4), np.complex64)
    for k in range(NCORES):
        out[:, :, :, k * TS:(k + 1) * TS] = _out_to_complex(res[k]["outp"])
    return out


# revision 49
# speedup vs baseline: 1.0295x; 1.0295x over previous
"""Wilson-Dirac operator on Trainium2, 8 NeuronCores, T-axis domain decomposition.

v4: DVE + TensorE hybrid. The baseline (v2) ran everything on the DVE
(99% busy, 627us). v4 keeps only the per-site work that genuinely needs a
two-tensor elementwise engine on the DVE (spin projection `proj`, SU(3)
component products `products`, and one half-reduction add per direction),
and moves ALL constant-coefficient linear algebra (color reduction over a,
the complex re/im combination, the spin expansion out[s] += d*m[e], the
y-shift of the forward-y hop term, and the mass term) onto the idle
TensorEngine as accumulating identity/permutation-weight matmuls into PSUM:

    out_psum[(s,ri) region][row, c*S + t*Z + z]  +=  sigma * P[row, plane-AP]

with lhsT in {+I, -I, +Yshift, -Yshift, 4.5*I} (contract over the partition
dim = lattice rows; the plane selection/z-shift/swizzle lives in the moving
operand's AP). Each (s,ri) region is 3*S = 432 fp32 = one PSUM bank; the
24-plane output accumulator occupies exactly the 8 banks. ScalarE (also
idle in v2) evacuates PSUM fp32 -> SBUF fp16 per region, then the result is
DMA'd out. fp32 PSUM accumulation also improves precision vs the fp16
RMW chain of v2.

Per direction the PE consumes P via 4 accumulating matmuls per (s,ri)
region: 2 signed (th,tu)-groups x 2 partial sums {P01 = a0+a1 (DVE), Pa2}.
DIRS_NOBSUM directions skip the DVE half-bsum and use 6 matmuls/region
(3 a-terms per group); the set is tuned so DVE busy ~= PE busy (~320us
each, overlapped to a ~360us wall vs the 628us all-DVE baseline).

Shift handling: t via inline halos; x via DRAM row reloads; BOTH y hops
via permutation weights on the PE (fwd: out rows shifted; bwd: host ships
a y-rolled gauge slot and the PE shifts the product rows back) - no shift
DMAs remain; z-fwd via a z-offset in the PE moving AP (split main+wrap
matmuls); z-bwd via an h-plane copy on the otherwise idle ScalarE.

Other scheduling tricks: psi/gauge/x-row loads prefetched one x-block
ahead (the sync queue otherwise stalls loads behind the out-DMA, which
waits on the PSUM evacuation); 48 junk matmuls at startup spin the PE
through the HAM activity window so real matmuls run at 2.4 GHz; the last
block orders PE-heavy dirs first to shrink the end-of-kernel PE backlog;
P tiles triple-buffered so the DVE can run ~3 dirs ahead of the PE."""

import numpy as np

# ---------------------------------------------------------------- constants
X = Y = 24
Z = 24
T = 48
NCORES = 8
TS = T // NCORES          # 6 t-slices per core
TH = TS + 2               # psi t slots (with halo both sides)
TG = TS + 1               # gauge t slots (halo at t0-1)
S = TS * Z                # 144: work-plane span [t,z]
PP = TH * Z               # 192: psi plane span
GP = TG * Z               # 168: gauge plane span
XY = X * Y
MASSP4 = 4.5
RMAX = 120                # max rows per x-block (NXC*Y)

# h_j = psi[j] + c_j * psi[B_j]; expansion: out[0]+=m[0], out[1]+=m[1],
# out[2] += d0*m[e0], out[3] += d1*m[e1].  Backward: c -> -c, d -> -d.
DIRSPEC = {
    0: dict(B=(3, 2), c=(-1j, -1j), e=(1, 0), d=(+1j, +1j)),
    1: dict(B=(3, 2), c=(-1, +1),   e=(1, 0), d=(+1, -1)),
    2: dict(B=(2, 3), c=(-1j, +1j), e=(0, 1), d=(+1j, -1j)),
    3: dict(B=(2, 3), c=(+1, +1),   e=(0, 1), d=(+1, +1)),
}

# processing order; x-dirs last so their DRAM row loads prefetch; (2,-1)
# delayed so its ScalarE h z-shift (queued behind the previous block's
# evacuations) has latency headroom.
DIRS_ORDER = [(2, +1), (3, +1), (3, -1), (2, -1),
              (1, +1), (1, -1), (0, +1), (0, -1)]
# last block: PE-heavy dirs first so the PE backlog at kernel end is small
# (the tail is PE finishing after the DVE's last op).
DIRS_ORDER_LAST = [(2, +1), (3, +1), (3, -1), (2, -1),
                   (0, +1), (1, +1), (1, -1), (0, -1)]
# dirs whose a-reduction happens fully on the PE (6 matmuls/region instead
# of 4) to offload the DVE.  Tuned against engine-busy split.
DIRS_NOBSUM = {(3, +1), (3, -1), (0, +1), (2, -1)}

_CACHE = {}


def _split_waits_json(raw: bytes) -> bytes:
    """Walrus here allows only ONE sync-wait per instruction. Keep the last
    wait on the instruction, hoist the rest onto NoOps inserted immediately
    before it (same engine, semaphores monotonic => exact)."""
    import json
    bj = json.loads(raw)
    nid = 0
    for fn in bj.get("functions", []):
        for bb in fn.get("blocks", []):
            out = []
            changed = False
            for inst in bb.get("instructions", []):
                si = inst.get("sync_info")
                ow = (si or {}).get("on_wait") or []
                if len(ow) > 1:
                    changed = True
                    for w in ow[:-1]:
                        nid += 1
                        out.append({
                            "engine": inst["engine"], "ins": [], "outs": [],
                            "name": f"WSPL-{nid}", "opcode": "NoOp",
                            "sync_info": {"on_update": [], "on_wait": [w]},
                        })
                    si["on_wait"] = [ow[-1]]
                out.append(inst)
            if changed:
                bb["instructions"] = out
    return json.dumps(bj).encode()


def _install_json_wait_fix():
    import concourse.bass as bass
    if getattr(bass.Bass, "_wd_wait_fix", False):
        return
    orig = bass.Bass.to_json_bytes

    def patched(self, *a, **k):
        return _split_waits_json(orig(self, *a, **k))

    bass.Bass.to_json_bytes = patched
    bass.Bass._wd_wait_fix = True


def _dir_terms(mu, sgn):
    """Per (s, ri) output region: list of (sigma, j, th, tu) matmul terms.

    m_re[j] = P[j,0,0] + f*P[j,1,1];  m_im[j] = P[j,1,0] - f*P[j,0,1]
    (f = +1 fwd / -1 bwd; P[j,th,tu] is the a-summed product with h-part th
    and U-part tu).  Regions (s,ri):
      s in {0,1}: += m[j=s, ri]
      s = 2+si:   j = e[si], dv = +-d[si]:
        dv real:  += sign(dv) * m[j, ri]
        dv imag:  region ri=0 += -sg*m_im[j]; region ri=1 += +sg*m_re[j]
    """
    spec = DIRSPEC[mu]
    fwd = sgn > 0
    f = 1 if fwd else -1
    out = {}

    def m_terms(j, ri, sigma):
        if ri == 0:   # m_re
            return [(sigma, j, 0, 0), (sigma * f, j, 1, 1)]
        else:         # m_im
            return [(sigma, j, 1, 0), (-sigma * f, j, 0, 1)]

    for s in (0, 1):
        for ri in (0, 1):
            out[(s, ri)] = m_terms(s, ri, +1)
    for si in (0, 1):
        s = 2 + si
        j = spec["e"][si]
        dv = spec["d"][si] if fwd else -spec["d"][si]
        if dv.imag == 0.0:
            sigma = 1 if dv.real > 0 else -1
            for ri in (0, 1):
                out[(s, ri)] = m_terms(j, ri, sigma)
        else:
            sg = 1 if dv.imag > 0 else -1
            out[(s, 0)] = m_terms(j, 1, -sg)
            out[(s, 1)] = m_terms(j, 0, +sg)
    return out


def build_module(NXC=5):
    import concourse.bass as bass
    import concourse.mybir as mybir
    from concourse.ap import AP
    from concourse.mybir import AluOpType
    from concourse.tile import TileContext

    _install_json_wait_fix()

    F16 = mybir.dt.float16
    F32 = mybir.dt.float32

    nc = bass.Bass()
    fh = nc.declare_dram_parameter("fh", [XY, 24 * PP], F16, isOutput=False)
    # gauge packed row-major: per row the 5 slots [mu0..mu3, mu1 y-rolled]
    # so one DMA per block fetches all local gauge.
    gg = nc.declare_dram_parameter("gg", [XY, 5 * 18 * GP], F16, isOutput=False)
    gx = nc.declare_dram_parameter("gx", [XY, 18 * GP], F16, isOutput=False)
    # weight matrices [RMAX, RMAX]: [I+, I-, Yf+, Yf-, 4.5I, Yb+, Yb-]
    wc = nc.declare_dram_parameter("wc", [RMAX, 7 * RMAX], F16, isOutput=False)
    outp = nc.declare_dram_parameter("outp", [XY, 24 * S], F16, isOutput=True)

    def sap(t, off, dims):
        return AP(t.tensor, t.offset + off, [list(t.ap[0])] + [list(d) for d in dims])

    def psap(t, off, dims, rows):
        ap0 = [list(t.ap[0])[0], rows]
        return AP(t.tensor, t.offset + off, [ap0] + [list(d) for d in dims])

    with TileContext(nc) as tc:
        ctx_pool = tc.tile_pool(name="work", bufs=1)
        pool = ctx_pool.__enter__()
        ctx_psum = tc.tile_pool(name="acc", bufs=1, space="PSUM")
        ppool = ctx_psum.__enter__()
        V = nc.vector
        A = AluOpType

        # persistent weight tiles
        wt = pool.tile([RMAX, 7 * RMAX], F16, tag="wc", bufs=1)
        nc.scalar.dma_start(out=wt[:], in_=wc[:])
        warm = {"todo": True}

        def W(idx, rows):
            # lhsT [K=rows, M=rows] slice of weight matrix idx
            return AP(wt.tensor, wt.offset + idx * RMAX,
                      [[list(wt.ap[0])[0], rows], [1, rows]])

        def dma(out, in_):
            nc.sync.dma_start(out=out, in_=in_)

        def load_rows(tag, src, drow, nelem, bufs, base, nrows, eng=None,
                      delay_on=None):
            tl = pool.tile([nrows, nelem], F16, tag=tag, bufs=bufs, name=tag)
            if delay_on is not None:
                # tiny dummy copy: makes this tile's DMA wait (WAW) until
                # `delay_on` has landed, keeping SDMA bandwidth focused on
                # the critical first loads.
                nc.scalar.copy(out=tl[0:1, 0:8], in_=delay_on[0:1, 0:8])
            rs = (base + drow) % XY
            def issue(o, i):
                if eng is None:
                    dma(o, i)
                else:
                    eng(out=o, in_=i)
            if rs + nrows <= XY:
                issue(tl[:], src(rs, rs + nrows))
            else:
                n1 = XY - rs
                issue(tl[0:n1], src(rs, XY))
                issue(tl[n1:nrows], src(0, nrows - n1))
            return tl

        def load_gauge(gl, base, mus, eng=None, delay_on=None):
            nr = min(NXC * Y, XY - base)
            for mu in mus:
                c0 = mu * 18 * GP
                gl[mu] = load_rows(f"g{mu}",
                                   lambda a, b, c=c0: gg[a:b, c:c + 18 * GP],
                                   0, 18 * GP, 2, base, nr, eng,
                                   delay_on=delay_on)
            return gl

        def load_main(base):
            """psi + local gauge slots (separate DMAs, consumption order so
            the first directions' data lands first)."""
            nr = min(NXC * Y, XY - base)
            psi = load_rows("psi", lambda a, b: fh[a:b], 0, 24 * PP, 2, base, nr)
            return psi, load_gauge({}, base, (2, 3, 1, 4, 0))

        def load_xtiles(base, eng=None, delay_on=None):
            nr = min(NXC * Y, XY - base)
            return (
                load_rows("pxf", lambda a, b: fh[a:b], -Y, 24 * PP, 2, base,
                          nr, eng, delay_on=delay_on),
                load_rows("pxb", lambda a, b: fh[a:b], +Y, 24 * PP, 2, base,
                          nr, eng, delay_on=delay_on),
                load_rows("gxf", lambda a, b: gx[a:b], -Y, 18 * GP, 2, base,
                          nr, eng, delay_on=delay_on),
            )

        xpre = None
        mpre = None
        for x0 in range(0, X, NXC):
            nx = min(NXC, X - x0)
            R = nx * Y
            r0 = x0 * Y

            # main + x-shifted loads: prefetched one block ahead so the sync
            # queue issues them before it blocks on the previous block's
            # out-DMA.  Block 0: load ONLY psi+g2 before the first DVE ops
            # (the first DVE instruction conservatively waits on every DMA
            # issued before it); everything else is emitted after front(0).
            xt = {}
            defer_loads = mpre is None
            if defer_loads:
                nr0 = min(NXC * Y, XY - r0)
                psi_al = pool.tile([nr0, 24 * PP], F16, tag="psi", bufs=2,
                                   name="psi")
                h0 = nr0 // 2
                nc.sync.dma_start(out=psi_al[0:h0], in_=fh[r0:r0 + h0])
                nc.scalar.dma_start(out=psi_al[h0:nr0],
                                    in_=fh[r0 + h0:r0 + nr0])
                gl = load_gauge({}, r0, (2,))
            else:
                psi_al, gl = mpre
                xt["v"] = xpre
            dirs = DIRS_ORDER_LAST if x0 + NXC >= X else DIRS_ORDER
            if not defer_loads:
                if x0 + NXC < X:
                    mpre = load_main((x0 + NXC) * Y)
                    xpre = load_xtiles((x0 + NXC) * Y)
                else:
                    mpre = xpre = None

            # PSUM accumulator regions: one bank per (s, ri)
            acc = {}
            for s in range(4):
                for ri in (0, 1):
                    acc[(s, ri)] = ppool.tile([RMAX, 512], F32,
                                              name=f"ps{s}{ri}",
                                              tag=f"ps{s}{ri}", bufs=1)
            # emit matmuls dir-major (PE consumes P tiles as DVE finishes
            # them); start/stop flags from precomputed per-region totals.
            n_per_dir = {}
            for mu_, sg_ in DIRS_ORDER:
                n = (2 if (mu_, sg_) in DIRS_NOBSUM else 0) + 4
                if mu_ == 2 and sg_ == +1:
                    n *= 2                       # z-fwd main+wrap split
                n_per_dir[(mu_, sg_)] = n
            region_total = 1 + sum(n_per_dir.values())   # +1 for mass
            mm_count = {k: 0 for k in acc}

            def emit_mm(key, widx, rhs, oap):
                i = mm_count[key]
                mm_count[key] = i + 1
                nc.tensor.matmul(oap, W(widx, rhs.ap[0][1]), rhs,
                                 start=(i == 0), stop=(i == region_total - 1))

            if warm.pop("todo", False):
                # junk matmuls while the first loads land: spin the PE past
                # the HAM 3.4us activity window so real matmuls run at
                # 2.4 GHz.  Results are discarded - the first real matmul per
                # region has start=True, which clears the bank.
                for w in range(48):
                    reg = acc[(w % 4, (w // 4) % 2)]
                    nc.tensor.matmul(
                        psap(reg, 0, [[1, RMAX]], RMAX),
                        W(0, RMAX), AP(wt.tensor, wt.offset,
                                       [list(wt.ap[0]), [1, RMAX]]),
                        start=True, stop=True, skip_group_check=True)

            # ---------------- op emitters ----------------------------------
            def proj(psi_t, toff, B, cj):
                """h[j,b,ri] = psi[A=j] + c_j psi[B_j]; returns h tile."""
                ht = pool.tile([R, 12 * S], F16, tag="h", bufs=3)
                jB = (B[1] - B[0]) * 6 * PP
                if cj[0] == cj[1] and cj[0].imag == 0.0:
                    op = A.add if cj[0].real > 0 else A.subtract
                    V.tensor_tensor(
                        sap(ht, 0, [[6 * S, 2], [S, 6], [1, S]]),
                        sap(psi_t, toff, [[6 * PP, 2], [PP, 6], [1, S]]),
                        sap(psi_t, B[0] * 6 * PP + toff, [[jB, 2], [PP, 6], [1, S]]),
                        op)
                elif cj[0] == cj[1]:
                    sg = cj[0].imag > 0
                    V.tensor_tensor(
                        sap(ht, 0, [[6 * S, 2], [2 * S, 3], [1, S]]),
                        sap(psi_t, toff, [[6 * PP, 2], [2 * PP, 3], [1, S]]),
                        sap(psi_t, B[0] * 6 * PP + PP + toff,
                            [[jB, 2], [2 * PP, 3], [1, S]]),
                        A.subtract if sg else A.add)
                    V.tensor_tensor(
                        sap(ht, S, [[6 * S, 2], [2 * S, 3], [1, S]]),
                        sap(psi_t, PP + toff, [[6 * PP, 2], [2 * PP, 3], [1, S]]),
                        sap(psi_t, B[0] * 6 * PP + toff,
                            [[jB, 2], [2 * PP, 3], [1, S]]),
                        A.add if sg else A.subtract)
                    return ht
                else:
                    for j in (0, 1):
                        c = cj[j]
                        ab = j * 6 * PP + toff
                        bb_ = B[j] * 6 * PP + toff
                        if c.imag == 0.0:
                            op = A.add if c.real > 0 else A.subtract
                            V.tensor_tensor(
                                sap(ht, j * 6 * S, [[S, 6], [1, S]]),
                                sap(psi_t, ab, [[PP, 6], [1, S]]),
                                sap(psi_t, bb_, [[PP, 6], [1, S]]), op)
                        else:
                            sg = c.imag > 0
                            # h_re = psiA_re -+ psiB_im ; h_im = psiA_im +- psiB_re
                            V.tensor_tensor(
                                sap(ht, j * 6 * S, [[2 * S, 3], [1, S]]),
                                sap(psi_t, ab, [[2 * PP, 3], [1, S]]),
                                sap(psi_t, bb_ + PP, [[2 * PP, 3], [1, S]]),
                                A.subtract if sg else A.add)
                            V.tensor_tensor(
                                sap(ht, j * 6 * S + S, [[2 * S, 3], [1, S]]),
                                sap(psi_t, ab + PP, [[2 * PP, 3], [1, S]]),
                                sap(psi_t, bb_, [[2 * PP, 3], [1, S]]),
                                A.add if sg else A.subtract)
                return ht

            def su3_front(g_t, gtoff, transposed, ht, half_bsum):
                """products (+ optional a0+=a1 half reduction) into P tile."""
                pt = pool.tile([R, 72 * S], F16, tag="P", bufs=3)
                if transposed:
                    gdims = [[3 * GP, 3], [GP, 3], [1, S]]     # read U[b,a]
                else:
                    gdims = [[GP, 3], [3 * GP, 3], [1, S]]     # read U[a,b]
                for j in (0, 1):
                    for th in (0, 1):
                        for tu in (0, 1):
                            V.tensor_tensor(
                                sap(pt, (j * 36 + th * 18 + tu * 9) * S,
                                    [[3 * S, 3], [S, 3], [1, S]]),
                                sap(g_t, tu * 9 * GP + gtoff, gdims),
                                sap(ht, (j * 6 + th) * S, [[2 * S, 3], [0, 3], [1, S]]),
                                A.mult)
                if half_bsum:
                    bdims = [[9 * S, 8], [1, 3 * S]]
                    V.tensor_tensor(sap(pt, 0, bdims), sap(pt, 0, bdims),
                                    sap(pt, 3 * S, bdims), A.add)
                return pt

            def zshift(src_t, nplanes, dz, tag):
                """dst[t,z] = src[t, z+dz] (periodic), dz in {-1,+1}.
                Runs on ScalarE (idle) to keep the DVE free."""
                dt_ = pool.tile([R, nplanes * S], F16, tag="h", bufs=3)
                C = nc.scalar.copy
                if dz == +1:
                    C(sap(dt_, 0, [[S, nplanes], [Z, TS], [1, Z - 1]]),
                      sap(src_t, 1, [[S, nplanes], [Z, TS], [1, Z - 1]]))
                    C(sap(dt_, Z - 1, [[S, nplanes], [Z, TS], [1, 1]]),
                      sap(src_t, 0, [[S, nplanes], [Z, TS], [1, 1]]))
                else:
                    C(sap(dt_, 1, [[S, nplanes], [Z, TS], [1, Z - 1]]),
                      sap(src_t, 0, [[S, nplanes], [Z, TS], [1, Z - 1]]))
                    C(sap(dt_, 0, [[S, nplanes], [Z, TS], [1, 1]]),
                      sap(src_t, Z - 1, [[S, nplanes], [Z, TS], [1, 1]]))
                return dt_

            # ---------------- PE accumulation ------------------------------
            def queue_dir_mms(pt, mu, sgn):
                """emit the per-region accumulating matmuls for direction."""
                terms = _dir_terms(mu, sgn)
                half = (mu, sgn) not in DIRS_NOBSUM
                # y hops: the row shift is a permutation in the weights
                wbase = 2 if (mu, sgn) == (1, +1) else \
                        5 if (mu, sgn) == (1, -1) else 0
                zs = (mu == 2 and sgn == +1)      # z-fwd: shift via rhs AP
                for (s, ri), tl in terms.items():
                    oreg = acc[(s, ri)]
                    for (sigma, j, th, tu) in tl:
                        base = (j * 36 + th * 18 + tu * 9) * S
                        widx = wbase + (0 if sigma > 0 else 1)
                        aoffs = (0, 6 * S) if half else (0, 3 * S, 6 * S)
                        for ao in aoffs:
                            if not zs:
                                rhs = sap(pt, base + ao, [[S, 3], [1, S]])
                                oap = psap(oreg, 0, [[S, 3], [1, S]], R)
                                emit_mm((s, ri), widx, rhs, oap)
                            else:
                                # out[.., z] += sigma * P[.., z-1 (mod Z)]
                                rhs = sap(pt, base + ao, [[Z, 3 * TS], [1, Z - 1]])
                                oap = psap(oreg, 1, [[Z, 3 * TS], [1, Z - 1]], R)
                                emit_mm((s, ri), widx, rhs, oap)
                                rhsw = sap(pt, base + ao + Z - 1,
                                           [[Z, 3 * TS], [1, 1]])
                                oapw = psap(oreg, 0, [[Z, 3 * TS], [1, 1]], R)
                                emit_mm((s, ri), widx, rhsw, oapw)

            def queue_mass():
                for s in range(4):
                    for ri in (0, 1):
                        rhs = sap(psi_al, s * 6 * PP + ri * PP + Z,
                                  [[2 * PP, 3], [1, S]])
                        oap = psap(acc[(s, ri)], 0, [[S, 3], [1, S]], R)
                        emit_mm((s, ri), 4, rhs, oap)

            # ---------------- direction pipeline ---------------------------
            st = {}

            def make_h(i):
                mu, sgn = dirs[i]
                spec = DIRSPEC[mu]
                fwd = sgn == +1
                cj = spec["c"] if fwd else tuple(-v for v in spec["c"])
                if mu == 2:
                    ht = proj(psi_al, Z, spec["B"], cj)
                    if not fwd:
                        ht = zshift(ht, 12, +1, "hsh")
                elif mu == 3:
                    ht = proj(psi_al, 0 if fwd else 2 * Z, spec["B"], cj)
                elif mu == 1:
                    # both y hops computed at the source row; the row shift
                    # happens in the PE weights (Yf/Yb permutations)
                    ht = proj(psi_al, Z, spec["B"], cj)
                else:
                    psi_xf, psi_xb, _ = xt["v"]
                    ht = proj(psi_xf if fwd else psi_xb, Z, spec["B"], cj)
                st[("h", i)] = ht

            def front(i):
                mu, sgn = dirs[i]
                fwd = sgn == +1
                if mu == 2:
                    g_t, gtoff = gl[2], Z
                elif mu == 3:
                    g_t, gtoff = gl[3], (0 if fwd else Z)
                elif mu == 1:
                    # bwd-y reads the host-y-rolled gauge (U at row y-1)
                    g_t, gtoff = (gl[1] if fwd else gl[4]), Z
                else:
                    g_t, gtoff = (xt["v"][2] if fwd else gl[0]), Z
                half = (mu, sgn) not in DIRS_NOBSUM
                pt = su3_front(g_t, gtoff, fwd, st[("h", i)], half)
                queue_dir_mms(pt, mu, sgn)

            # front(0) ASAP so the PE gets work early; make_h(5) hoisted to
            # give its y-shift row DMAs (scalar queue) latency headroom.
            queue_mass()
            done_h = set()

            def ensure_h(i):
                if i not in done_h:
                    make_h(i)
                    done_h.add(i)

            ensure_h(0)
            front(0)
            if defer_loads:
                load_gauge(gl, r0, (3, 1), eng=nc.scalar.dma_start)
                load_gauge(gl, r0, (4, 0))
                xt["v"] = load_xtiles(r0)
                if x0 + NXC < X:
                    mpre = load_main((x0 + NXC) * Y)
                    xpre = load_xtiles((x0 + NXC) * Y)
                else:
                    mpre = xpre = None
            ensure_h(1)
            front(1)
            ensure_h(3)   # lead time for the ScalarE h z-shift of (2,-1)
            for i in range(2, 8):
                ensure_h(i)
                front(i)

            # ---------------- evacuate PSUM -> SBUF fp16 -------------------
            out_t = pool.tile([R, 24 * S], F16, tag="out", bufs=1)
            for s in range(4):
                for ri in (0, 1):
                    nc.scalar.copy(
                        out=sap(out_t, (s * 6 + ri) * S, [[2 * S, 3], [1, S]]),
                        in_=psap(acc[(s, ri)], 0, [[S, 3], [1, S]], R))

            nc.sync.dma_start(out=outp[r0:r0 + R], in_=out_t[:])
        ctx_psum.__exit__(None, None, None)
        ctx_pool.__exit__(None, None, None)
    return nc


# ---------------------------------------------------------------- host side
def _prep_core_inputs(fv, gv, t0):
    """fv: [X,Y,Z,T,3,4,2] f32 (c,s,ri). gv: [4,X,Y,Z,T,3,3,2] (r,c,ri).
    Returns fh [XY, 24*(TH*Z)] planes (s,c,ri) layout [t,z], and
    gg [4, XY, 18*(TG*Z)] planes (ri,r,c) of -0.5*U, both fp16."""
    Tl = T
    slots = [(t0 - 1) % Tl] + [(t0 + i) % Tl for i in range(TS)] + [(t0 + TS) % Tl]
    f = fv[:, :, :, slots]                       # [X,Y,Z,TH,c,s,ri]
    f = f.transpose(0, 1, 5, 4, 6, 3, 2)         # [X,Y,s,c,ri,TH,Z]
    fhn = np.ascontiguousarray(f, dtype=np.float16).reshape(XY, 24 * PP)
    gslots = [(t0 - 1 + i) % Tl for i in range(TG)]
    g = gv[:, :, :, :, gslots]                   # [4,X,Y,Z,TG,r,c,ri]
    g = g.transpose(0, 1, 2, 7, 5, 6, 4, 3)      # [4,X,Y,ri,r,c,TG,Z]
    ggn = np.ascontiguousarray(g, dtype=np.float32)
    ggn *= -0.5
    # slot 4: mu1 gauge y-rolled (+1): g1y[x, y] = g1[x, y-1]
    g1y = np.roll(ggn[1], 1, axis=1)
    ggn = np.concatenate([ggn, g1y[None]], axis=0).astype(np.float16)
    ggn = ggn.reshape(5, XY, 18 * GP)
    # pack row-major: [XY, 5 slots x 18*GP]; plus standalone mu0 for x-loads
    ggp = np.ascontiguousarray(ggn.transpose(1, 0, 2)).reshape(XY, 5 * 18 * GP)
    gxn = np.ascontiguousarray(ggn[0])
    return fhn, ggp, gxn


def _make_weights():
    """[RMAX, 7*RMAX] fp16: [I+, I-, Yf+, Yf-, 4.5I, Yb+, Yb-].
    Yf: W[p_in, p_out] = 1 iff p_out = p_in + 1 (mod Y, per x-line):
    out[y] += P[y-1] (fwd-y hop).  Yb: p_out = p_in - 1 (bwd-y hop)."""
    eye = np.eye(RMAX, dtype=np.float16)
    ysh = np.zeros((RMAX, RMAX), dtype=np.float16)
    for g in range(RMAX // Y):
        b = g * Y
        for y in range(Y):
            ysh[b + y, b + (y + 1) % Y] = 1.0
    ybk = ysh.T.copy()
    return np.concatenate([eye, -eye, ysh, -ysh,
                           np.float16(MASSP4) * eye, ybk, -ybk], axis=1)


def _out_to_complex(o):
    o = o.astype(np.float32).reshape(X, Y, 4, 3, 2, TS, Z)   # [X,Y,s,c,ri,t,z]
    o = o.transpose(0, 1, 6, 5, 3, 2, 4)                     # [X,Y,Z,t,c,s,ri]
    return (o[..., 0] + 1j * o[..., 1]).astype(np.complex64)


def _build_in_maps(field, gauge_field):
    fv = np.ascontiguousarray(field).view(np.float32).reshape(X, Y, Z, T, 3, 4, 2)
    gv = np.ascontiguousarray(gauge_field).view(np.float32).reshape(4, X, Y, Z, T, 3, 3, 2)
    wcn = _make_weights()
    in_maps = []
    for k in range(NCORES):
        fhn, ggp, gxn = _prep_core_inputs(fv, gv, k * TS)
        in_maps.append({"fh": fhn, "gg": ggp, "gx": gxn, "wc": wcn})
    return in_maps


def kernel(field, gauge_field):
    from concourse.bass_utils import run_bass_kernel_spmd

    key = "full"
    if key not in _CACHE:
        _CACHE[key] = build_module()
    nc = _CACHE[key]

    in_maps = _build_in_maps(field, gauge_field)
    res = run_bass_kernel_spmd(nc, in_maps, list(range(NCORES))).results

    out = np.empty((X, Y, Z, T, 3, 4), np.complex64)
    for k in range(NCORES):
        out[:, :, :, k * TS:(k + 1) * TS] = _out_to_complex(res[k]["outp"])
    return out
